# revision 5
# baseline (speedup 1.0000x reference)
"""Contrastive-loss kernel for 8 Trainium2 NeuronCores.

Strategy (hardcoded for emb_i/emb_j of shape [50, 524288] float32):
  - Host: concat emb_i/emb_j into reps [100, 524288]; shard the feature
    (K) dimension 8 ways (65536 per core); pre-permute each shard into a
    [128, 512*100] layout so each device DMA is fully contiguous and K
    lands on the partition axis for the PE matmul.
  - Device (per core): stream the shard in f32 (25.6 MB HBM traffic),
    cast to bf16, accumulate the partial gram matrix G = X @ X.T
    ([100, 100]) in PSUM over 512 K-chunks of 128.
  - AllReduce the partial grams across the 8 cores, then run the loss
    epilogue replicated on every core: normalize via rsqrt(diag), exp,
    masked row-reductions (masks are host-provided constants), log, and
    a partition-axis sum via a [100,1]x[100,1] matmul.
  - Output: scalar loss (core 0's copy).
"""

import os
import sys
import types

import numpy as np

BATCH = 50
M = 2 * BATCH            # 100 rows in the gram matrix
DIM = 524288
N_CORES = 8
D_LOC = DIM // N_CORES   # 65536 features per core
P = 128                  # partitions (K-chunk size)
K_CHUNKS = D_LOC // P    # 512 chunks per core
TILE_CH = 32             # K-chunks per DMA tile
N_TILES = K_CHUNKS // TILE_CH
TILE_W = TILE_CH * M     # free width of one DMA tile (3200 f32)
TEMP = 0.5
GROUP = 5
LOSS_DIV = 91.0


def _install_ntff_hook():
    """Register the axon NTFF profile hook if the image lacks antenv.axon_hooks.

    Without this, run_bass_kernel_spmd(trace=True) silently skips profiling.
    Harmless if profiling is never requested.
    """
    try:
        import antenv.axon_hooks  # noqa: F401

        return
    except ImportError:
        pass
    try:
        import antenv
        from trn_agent_boot.trn_boot import _ntff_profile_via_ctypes

        mod = types.ModuleType("antenv.axon_hooks")
        mod._hook = _ntff_profile_via_ctypes("/opt/axon/libaxon_pjrt.so")
        mod.get_axon_ntff_profile_hook = lambda: mod._hook
        mod.set_axon_ntff_profile_hook = lambda h: setattr(mod, "_hook", h)
        antenv.axon_hooks = mod
        sys.modules["antenv.axon_hooks"] = mod
    except Exception:
        pass


_install_ntff_hook()

_NC = None        # cached compiled Bass module
LAST = None       # last BassKernelResults (exec_time_ns etc.), for test harnesses


def _build_masks():
    """Host-side constant masks for the loss epilogue (all [100, 100] f32)."""
    idx = np.arange(M)
    g = (idx % BATCH) // GROUP
    mnom = np.zeros((M, M), dtype=np.float32)
    for a in range(M):
        base = g[a] * GROUP
        mnom[a, base : base + GROUP] = 1.0
        mnom[a, BATCH + base : BATCH + base + GROUP] = 1.0
    mpos = np.zeros((M, M), dtype=np.float32)
    mpos[idx, (idx + BATCH) % M] = 1.0
    ident = np.eye(M, dtype=np.float32)
    return mnom, mpos, ident


def _build_bass(k_chunks=K_CHUNKS, tile_ch=TILE_CH):
    import concourse.bacc as bacc
    import concourse.mybir as mybir
    import concourse.tile as tile

    f32 = mybir.dt.float32
    bf16 = mybir.dt.bfloat16

    n_tiles = k_chunks // tile_ch
    tile_w = tile_ch * M

    nc = bacc.Bacc("TRN2", target_bir_lowering=False, debug=False,
                   num_devices=N_CORES)

    x = nc.dram_tensor("x", [P, k_chunks * M], f32, kind="ExternalInput")
    mnom = nc.dram_tensor("mnom", [M, M], f32, kind="ExternalInput")
    mpos = nc.dram_tensor("mpos", [M, M], f32, kind="ExternalInput")
    ident = nc.dram_tensor("ident", [M, M], f32, kind="ExternalInput")
    out = nc.dram_tensor("out", [1, 1], f32, kind="ExternalOutput")

    with tile.TileContext(nc) as tc:
        with tc.tile_pool(name="io", bufs=3) as io_pool, \
             tc.tile_pool(name="bf", bufs=3) as bf_pool, \
             tc.tile_pool(name="consts", bufs=1) as consts, \
             tc.tile_pool(name="epi", bufs=1) as epi, \
             tc.tile_pool(name="psum", bufs=2, space="PSUM") as psum_pool, \
             tc.tile_pool(name="dram", bufs=1, space="DRAM") as dram:

            g_psum = psum_pool.tile([M, M], f32)

            # Main streaming loop: DMA f32 tile, cast to bf16, accumulate
            # 32 gram-matmuls per tile into PSUM.
            for t in range(n_tiles):
                xt = io_pool.tile([P, tile_w], f32, tag="xt")
                nc.sync.dma_start(xt[:], x.ap()[:, t * tile_w : (t + 1) * tile_w])
                xb = bf_pool.tile([P, tile_w], bf16, tag="xb")
                # Alternate the cast between DVE and ACT so neither becomes
                # the bottleneck engine.
                if t % 2 == 0:
                    nc.vector.tensor_copy(xb[:], xt[:])
                else:
                    nc.scalar.copy(xb[:], xt[:])
                for j in range(tile_ch):
                    sl = xb[:, j * M : (j + 1) * M]
                    nc.tensor.matmul(
                        g_psum[:], lhsT=sl, rhs=sl,
                        start=(t == 0 and j == 0),
                        stop=(t == n_tiles - 1 and j == tile_ch - 1),
                    )

            # Partial gram -> DRAM bounce -> AllReduce(sum) across 8 cores.
            g_part = epi.tile([M, M], f32)
            nc.vector.tensor_copy(g_part[:], g_psum[:])
            cc_in = dram.tile([M, M], f32)
            cc_out = dram.tile([M, M], f32)
            nc.gpsimd.dma_start(cc_in[:], g_part[:])
            nc.gpsimd.collective_compute(
                "AllReduce",
                mybir.AluOpType.add,
                replica_groups=[list(range(N_CORES))],
                ins=[cc_in.opt()],
                outs=[cc_out.opt()],
            )
            g_sb = epi.tile([M, M], f32)
            nc.sync.dma_start(g_sb[:], cc_out[:])

            # Constants.
            mnom_sb = consts.tile([M, M], f32)
            mpos_sb = consts.tile([M, M], f32)
            ident_sb = consts.tile([M, M], f32)
            nc.sync.dma_start(mnom_sb[:], mnom.ap()[:])
            nc.sync.dma_start(mpos_sb[:], mpos.ap()[:])
            nc.sync.dma_start(ident_sb[:], ident.ap()[:])
            ones_sb = epi.tile([M, 1], f32)
            nc.vector.memset(ones_sb[:], 1.0)

            # diag[a] = G[a, a] via row-reduce of G * I.
            # (InstTensorTensorReduce crashes the exec unit on this runtime,
            # so use separate mul + reduce ops.)
            gi_tmp = epi.tile([M, M], f32)
            diag = epi.tile([M, 1], f32)
            nc.vector.tensor_mul(gi_tmp[:], g_sb[:], ident_sb[:])
            nc.vector.tensor_reduce(diag[:], gi_tmp[:],
                                    axis=mybir.AxisListType.X,
                                    op=mybir.AluOpType.add)
            # inv_n = 1/sqrt(diag)  (vector reciprocal + ACT sqrt: the
            # Rsqrt activation is banned for accuracy reasons).
            rd = epi.tile([M, 1], f32)
            nc.vector.reciprocal(rd[:], diag[:])
            inv_n = epi.tile([M, 1], f32)
            nc.scalar.sqrt(inv_n[:], rd[:])

            # sim = diag_scale(inv_n) @ G @ diag_scale(inv_n), done as
            # scale-rows -> PE transpose -> scale-rows (G is symmetric).
            h_sb = epi.tile([M, M], f32)
            nc.vector.tensor_scalar_mul(h_sb[:], g_sb[:], inv_n[:])
            ht_ps = psum_pool.tile([M, M], f32)
            nc.tensor.transpose(ht_ps[:], h_sb[:], ident_sb[:])
            sim_sb = epi.tile([M, M], f32)
            nc.vector.tensor_scalar_mul(sim_sb[:], ht_ps[:], inv_n[:])

            # E = exp(sim / T) with fused row-sum.
            e_sb = epi.tile([M, M], f32)
            rowsum = epi.tile([M, 1], f32)
            nc.scalar.activation(
                e_sb[:], sim_sb[:], mybir.ActivationFunctionType.Exp,
                scale=1.0 / TEMP, accum_out=rowsum[:],
            )

            # Masked row sums: nominator block sum and the positive term.
            tmp1 = epi.tile([M, M], f32)
            nom = epi.tile([M, 1], f32)
            nc.vector.tensor_mul(tmp1[:], e_sb[:], mnom_sb[:])
            nc.vector.tensor_reduce(nom[:], tmp1[:],
                                    axis=mybir.AxisListType.X,
                                    op=mybir.AluOpType.add)
            tmp2 = epi.tile([M, M], f32)
            epos = epi.tile([M, 1], f32)
            nc.vector.tensor_mul(tmp2[:], e_sb[:], mpos_sb[:])
            nc.vector.tensor_reduce(epos[:], tmp2[:],
                                    axis=mybir.AxisListType.X,
                                    op=mybir.AluOpType.add)

            # loss_partial = ln(rowsum - nom + epos) - ln(epos)
            # (the exp(sim[i,i]/T) self-terms cancel between the reference's
            # denominator and nominator).
            den = epi.tile([M, 1], f32)
            nc.vector.scalar_tensor_tensor(
                out=den[:], in0=nom[:], scalar=-1.0, in1=rowsum[:],
                op0=mybir.AluOpType.mult, op1=mybir.AluOpType.add,
            )
            den2 = epi.tile([M, 1], f32)
            nc.vector.tensor_add(den2[:], den[:], epos[:])
            lden = epi.tile([M, 1], f32)
            nc.scalar.activation(lden[:], den2[:], mybir.ActivationFunctionType.Ln)
            lpos = epi.tile([M, 1], f32)
            nc.scalar.activation(lpos[:], epos[:], mybir.ActivationFunctionType.Ln)
            lp = epi.tile([M, 1], f32)
            nc.vector.tensor_sub(lp[:], lden[:], lpos[:])

            # Partition-axis sum via PE: [100,1].T @ [100,1] -> [1,1].
            loss_ps = psum_pool.tile([1, 1], f32)
            nc.tensor.matmul(loss_ps[:], lhsT=lp[:], rhs=ones_sb[:],
                             start=True, stop=True)
            loss_sb = epi.tile([1, 1], f32)
            nc.scalar.mul(loss_sb[:], loss_ps[:], 1.0 / LOSS_DIV)
            nc.sync.dma_start(out.ap()[:], loss_sb[:])

    nc.compile()
    return nc


def kernel(emb_i: np.ndarray, emb_j: np.ndarray) -> np.ndarray:
    global _NC, LAST
    from concourse import bass_utils

    emb_i = np.ascontiguousarray(np.asarray(emb_i, dtype=np.float32))
    emb_j = np.ascontiguousarray(np.asarray(emb_j, dtype=np.float32))

    reps = np.concatenate([emb_i, emb_j], axis=0)          # [100, DIM]
    # Two-pass permute (cache-friendlier than one big gather):
    # repsT[d, m], then per-core [512, 128, 100] -> [128, 512, 100].
    repsT = np.ascontiguousarray(reps.T)                   # [DIM, 100]
    shards = []
    for c in range(N_CORES):
        s = repsT[c * D_LOC : (c + 1) * D_LOC]             # [65536, 100]
        y = np.ascontiguousarray(
            s.reshape(K_CHUNKS, P, M).transpose(1, 0, 2)
        ).reshape(P, K_CHUNKS * M)
        shards.append(y)

    mnom, mpos, ident = _build_masks()
    in_maps = [
        {"x": shards[c], "mnom": mnom, "mpos": mpos, "ident": ident}
        for c in range(N_CORES)
    ]

    if _NC is None:
        _NC = _build_bass()

    res = bass_utils.run_bass_kernel_spmd(
        _NC, in_maps, core_ids=list(range(N_CORES))
    )
    LAST = res
    loss = res.results[0]["out"][0, 0]
    return np.array(loss, dtype=np.float32)
